# revision 20
# baseline (speedup 1.0000x reference)
"""AdaptiveGraphSAGE on 8 TRN2 NeuronCores (Bass/Tile).

Sharding: dst nodes across 8 cores. Edge gathers via gpsimd.dma_gather
(bf16, transpose=True -> feature-major tiles), segment sums/maxes via
strided DVE tensor_reduce over host-padded degree buckets. The bucket
geometry is padded to a shared cross-core structure so all 8 cores run one
SPMD program. Layer-2 node table is rebuilt on device via AllGather; the
global mean pool uses an AllReduce. See NOTES.md for the full design.
"""

from collections import defaultdict

import numpy as np
import ml_dtypes

import concourse.bacc as bacc
import concourse.bass as bass
import concourse.mybir as mybir
import concourse.tile as tile
from concourse._compat import get_trn_type
from concourse.bass_utils import run_bass_kernel_spmd
from concourse.library_config import mlp as mlp_lib
from concourse.masks import make_identity

BF16 = np.float16  # fp16: 4x mantissa of bf16, range-safe here
NC = 8
P_PAD = 4            # pad each (dst, half) neighbor list to a multiple of this
TILE_SLOTS = 8192    # gather tile size in slots; multiple of 128
CHUNK = 2048         # max dsts per reduce run (even)
GI = 512             # max dma_gather num_idxs (SWDGE descriptor ring limit)
K0 = 100000          # bucket key encoding: key = L0*K0 + L1
NEG = -60000.0  # fp16-representable, below any real value
EPS = 1e-5
D = 128


# =============================================================== host prep

def _padP(n):
    return ((n + P_PAD - 1) // P_PAD) * P_PAD


def _softmax(a):
    e = np.exp(a - a.max())
    return e / e.sum()


def _wrap_idxs(stream):
    """[S] -> [128, S/16] int16, idx j at [j%16, j//16], replicated x8."""
    s = np.asarray(stream, np.int16).reshape(-1, 16).T
    return np.ascontiguousarray(np.tile(s, (8, 1)))


def _segments(all_keys, bucket_m, h):
    """Core-independent stream geometry for one half.

    Returns (segs, slots): segs = [(key, L, take, col, cursor)], where a seg
    is `take` dsts x window L starting at stream slot `cursor`, writing agg
    columns [col, col+take). Segs never cross a TILE_SLOTS boundary; takes
    and cols are even (DVE 4B alignment).
    """
    segs = []
    cursor = col = 0
    for k in all_keys:
        L = _seg_L(k, h)
        mtot = bucket_m[k]
        if L == 0:
            col += mtot
            continue
        i = 0
        while i < mtot:
            room = TILE_SLOTS - cursor % TILE_SLOTS
            take = min(CHUNK, mtot - i, room // L)
            take -= take % 2
            if take <= 0:
                cursor += room          # filler slots to tile boundary
                continue
            segs.append((k, L, take, col + i, cursor))
            cursor += take * L
            i += take
        col += mtot
    slots = max(-(-cursor // TILE_SLOTS) * TILE_SLOTS, TILE_SLOTS)
    return segs, slots


def _prep_layer_full(src_pos, dst, NL, S_in):
    HALF = 4 * S_in
    cores = []
    for c in range(NC):
        m = (dst >= c * NL) & (dst < (c + 1) * NL)
        sp = src_pos[m]
        dl = (dst[m] - c * NL).astype(np.int64)
        half = sp >= HALF
        cnt0 = np.bincount(dl[~half], minlength=NL)
        cnt1 = np.bincount(dl[half], minlength=NL)
        L0 = _padP(0) * 0 + ((cnt0 + P_PAD - 1) // P_PAD) * P_PAD
        L1 = ((cnt1 + P_PAD - 1) // P_PAD) * P_PAD
        key = L0.astype(np.int64) * K0 + L1
        perm = np.argsort(key, kind="stable")
        inv = np.empty(NL, np.int64)
        inv[perm] = np.arange(NL)
        cores.append(dict(sp=sp, dl=dl, half=half, cnt0=cnt0, cnt1=cnt1,
                          key=key, perm=perm, inv=inv))
    all_keys = sorted(set(np.concatenate(
        [np.unique(c["key"]) for c in cores]).tolist()))
    bucket_m = {}
    for k in all_keys:
        mx = max(int((c["key"] == k).sum()) for c in cores)
        bucket_m[k] = ((mx + 1) // 2) * 2
    ncols = sum(bucket_m.values())
    ncols_pad = -(-ncols // 128) * 128
    if ncols_pad > ncols:
        # trailing dummy bucket (L0=L1=0): occupies columns, no slots
        dk = -1  # sorts first; L decode of -1: guard below
        all_keys = [dk] + all_keys
        bucket_m[dk] = ncols_pad - ncols
    geo = {h: _segments(all_keys, bucket_m, h) for h in (0, 1)}
    return cores, all_keys, bucket_m, ncols_pad, geo


def _seg_L(k, h):
    if k == -1:
        return 0
    return (k // K0) if h == 0 else (k % K0)


def _canon_cols(core, all_keys, bucket_m):
    """Column of each real dst, in canonical order."""
    key_canon = core["key"][core["perm"]]
    cols = np.empty(len(key_canon), np.int64)
    col = p = 0
    for k in all_keys:
        nr = 0 if k == -1 else int((key_canon == k).sum())
        cols[p:p + nr] = col + np.arange(nr)
        p += nr
        col += bucket_m[k]
    assert p == len(cols)
    return cols


def _core_stream(core, all_keys, bucket_m, segs, slots, h, S_in):
    """One core's gather index stream (half-local table-row indices).

    Table layout: 8 blocks of S_in rows; block c = [zero; data (S_in-2); zero].
    Half h covers rows [4*S_in*h, 4*S_in*(h+1)); idx = row - 4*S_in*h.
    Pad idx 0 -> the half's leading zero row."""
    HALF = 4 * S_in
    pad_idx = 0
    out = np.full(slots, pad_idx, np.int64)

    sel = core["half"] == (h == 1)
    sp = core["sp"][sel]
    dl = core["dl"][sel]
    order = np.argsort(core["inv"][dl], kind="stable")
    vals = sp[order]
    if h == 1:
        vals = vals - HALF
    cnt_h = core["cnt0"] if h == 0 else core["cnt1"]
    cnt_canon = cnt_h[core["perm"]]

    key_canon = core["key"][core["perm"]]
    nreal = {k: (0 if k == -1 else int((key_canon == k).sum()))
             for k in all_keys}
    pstart = {}
    pp = 0
    for k in all_keys:
        pstart[k] = pp
        pp += nreal[k]
    done = defaultdict(int)
    vpos = 0
    ar = np.arange(10 ** 5)
    for (k, L, take, col, cursor) in segs:
        i = done[k]
        done[k] = i + take
        r0 = min(i, nreal[k])
        r1 = min(i + take, nreal[k])
        cn = np.zeros(take, np.int64)
        if r1 > r0:
            cn[r0 - i:r1 - i] = cnt_canon[pstart[k] + r0:pstart[k] + r1]
        nvt = int(cn.sum())
        if nvt == 0:
            continue
        mat = np.full((take, L), pad_idx, np.int64)
        mask = ar[None, :L] < cn[:, None]
        mat[mask] = vals[vpos:vpos + nvt]
        vpos += nvt
        out[cursor:cursor + take * L] = mat.reshape(-1)
    assert vpos == len(vals), (vpos, len(vals))
    return out


def host_prep(x, edge_index, params):
    N = x.shape[0]
    NL = N // NC
    src = np.asarray(edge_index[0], np.int64)
    dst = np.asarray(edge_index[1], np.int64)
    deg = np.bincount(dst, minlength=N)

    # table row of node n in the layer-1 (x) table: 8 blocks of NL+2 rows,
    # block c = [zero; x rows; zero]
    S0 = NL + 2
    row0 = (src // NL) * S0 + 1 + (src % NL)
    cores0, keys0, bm0, NCOLS0, geo0 = _prep_layer_full(row0, dst, NL, S0)
    S1 = NCOLS0 + 2
    g_row = np.empty(N, np.int64)
    for c in range(NC):
        g_row[c * NL + cores0[c]["perm"]] = (
            c * S1 + 1 + _canon_cols(cores0[c], keys0, bm0))
    cores1, keys1, bm1, NCOLS1, geo1 = _prep_layer_full(
        g_row[src], dst, NL, S1)

    prep = dict(N=N, NL=NL, deg=deg, g_row=g_row,
                layers=[
                    dict(cores=cores0, keys=keys0, bm=bm0, ncols=NCOLS0,
                         geo=geo0, S_in=S0),
                    dict(cores=cores1, keys=keys1, bm=bm1, ncols=NCOLS1,
                         geo=geo1, S_in=S1),
                ])

    per_core = [dict() for _ in range(NC)]
    TH0 = 8 * S0
    xb = np.asarray(x, np.float32).astype(BF16).reshape(NC, NL, D)
    xs = np.zeros((TH0, D), BF16)
    xm = np.full((TH0, D), BF16(NEG))
    for c in range(NC):
        xs[c * S0 + 1:c * S0 + 1 + NL] = xb[c]
        xm[c * S0 + 1:c * S0 + 1 + NL] = xb[c]
    for c in range(NC):
        per_core[c]["xs"] = xs
        per_core[c]["xm"] = xm

    for l, lay in enumerate(prep["layers"]):
        cores, keys, bm = lay["cores"], lay["keys"], lay["bm"]
        ncols, geo, S_in = lay["ncols"], lay["geo"], lay["S_in"]
        HALF = 4 * S_in
        for c in range(NC):
            core = cores[c]
            for h in (0, 1):
                segs, slots = geo[h]
                stream = _core_stream(core, keys, bm, segs, slots, h, S_in)
                w = _wrap_idxs(stream)                       # [128, slots/16]
                nt = slots // TILE_SLOTS
                per_core[c][f"idx_l{l}h{h}"] = np.ascontiguousarray(
                    w.reshape(128, nt, TILE_SLOTS // 16).transpose(1, 0, 2))
            # lin_r: table rows of own dsts in canonical order (+dummies=pad)
            ccols = _canon_cols(core, keys, bm)
            if l == 0:
                nn = c * NL + core["perm"]
                dst_row = (nn // NL) * S_in + 1 + (nn % NL)
            else:
                dst_row = g_row[c * NL + core["perm"]]
            pos = np.full(ncols, -1, np.int64)
            pos[ccols] = dst_row
            real = pos >= 0
            i0 = np.zeros(ncols, np.int64)          # half0 gather idx (pad 0)
            i1 = np.zeros(ncols, np.int64)          # half1 gather idx (pad 0)
            if c < NC // 2:
                i0[real] = pos[real]
            else:
                i1[real] = pos[real] - HALF
            per_core[c][f"idxr0_l{l}"] = _wrap_idxs(i0)
            per_core[c][f"idxr1_l{l}"] = _wrap_idxs(i1)
            # aux vectors, replicated over the 128 partitions
            invd = np.zeros(ncols, np.float32)
            d_here = deg[c * NL + core["perm"]]
            invd[ccols] = 1.0 / np.maximum(d_here, 1)
            per_core[c][f"invdeg_l{l}"] = np.ascontiguousarray(
                np.tile(invd.astype(BF16)[None, :], (128, 1)))
            if l == 0:
                mk = np.zeros(ncols, np.float32)
                mk[ccols] = (d_here > 0).astype(np.float32)
                per_core[c]["mask0"] = np.ascontiguousarray(
                    np.tile(mk.astype(BF16)[None, :], (128, 1)))
            else:
                mk = np.zeros(ncols, np.float32)
                mk[ccols] = 1.0
                per_core[c]["maskpool"] = np.ascontiguousarray(
                    np.tile(mk.astype(BF16)[None, :], (128, 1)))

    # ---- shared constants
    consts = {}
    for l, lp in enumerate(params["layers"]):
        w = _softmax(np.asarray(lp["aggr_w"], np.float64))
        for bi, bname in enumerate(("mean", "max")):
            p = lp[bname]
            tag = f"l{l}{'me' if bi == 0 else 'ma'}"
            consts[f"wl_{tag}"] = np.asarray(p["Wl"], np.float32).astype(BF16)
            consts[f"wr_{tag}"] = np.asarray(p["Wr"], np.float32).astype(BF16)
            consts[f"bl_{tag}"] = np.asarray(
                p["bl"], np.float32).reshape(D, 1).copy()
            s = np.asarray(p["gamma"], np.float64) / np.sqrt(
                np.asarray(p["rvar"], np.float64) + EPS)
            b = (np.asarray(p["beta"], np.float64)
                 - np.asarray(p["rmean"], np.float64) * s)
            consts[f"bns_{tag}"] = (w[bi] * s).astype(
                np.float32).reshape(D, 1).copy()
            consts[f"bnb_{tag}"] = (w[bi] * b).astype(
                np.float32).reshape(D, 1).copy()
    cls = params["classifier"]
    consts["cw1"] = np.asarray(cls[0]["W"], np.float32)                # [128,256]
    consts["cw2"] = np.ascontiguousarray(                              # [128,2,128]
        np.asarray(cls[1]["W"], np.float32).reshape(2, 128, 128)
        .transpose(1, 0, 2))
    consts["cw3"] = np.asarray(cls[2]["W"], np.float32)                # [128,2]
    for i in (0, 1):
        p = cls[i]
        s = np.asarray(p["gamma"], np.float64) / np.sqrt(
            np.asarray(p["rvar"], np.float64) + EPS)
        b = (np.asarray(p["beta"], np.float64)
             - np.asarray(p["rmean"], np.float64) * s
             + np.asarray(p["b"], np.float64) * s)
        if i == 0:      # [256] -> [128, 2] (col j = feats j*128..)
            consts["cs1"] = np.ascontiguousarray(
                s.astype(np.float32).reshape(2, 128).T)
            consts["cb1"] = np.ascontiguousarray(
                b.astype(np.float32).reshape(2, 128).T)
        else:
            consts["cs2"] = s.astype(np.float32).reshape(128, 1).copy()
            consts["cb2"] = b.astype(np.float32).reshape(128, 1).copy()
    consts["cb3"] = np.asarray(cls[2]["b"], np.float32).reshape(2, 1).copy()
    for c in range(NC):
        per_core[c].update(consts)
    return prep, per_core


# ============================================================ bass builder

def _dump_exit(nc, pool, out_d, src_ap):
    """Debug: write a [128,1] f32 view of src_ap to out[2:130] and stop."""
    t = pool.tile([128, 1], mybir.dt.float32, name="dbgdump")
    nc.vector.tensor_copy(out=t[:], in_=src_ap)
    nc.sync.dma_start(out=out_d[2:130].rearrange("(x o) -> x o", o=1),
                      in_=t[:])
    lgz = pool.tile([2, 1], mybir.dt.float32, name="dbglg")
    nc.vector.memset(lgz[:], 0.0)
    nc.sync.dma_start(out=out_d[0:2].rearrange("(x o) -> x o", o=1),
                      in_=lgz[:])


def build(nc, prep):
    import os
    STAGE = int(os.environ.get("BASS_STAGE", "99"))
    N, NL = prep["N"], prep["NL"]
    NCOLS0 = prep["layers"][0]["ncols"]
    NCOLS1 = prep["layers"][1]["ncols"]
    f32, bf16, i16 = mybir.dt.float32, mybir.dt.float16, mybir.dt.int16
    TH0 = 8 * (NL + 2)
    TH1 = 8 * (NCOLS0 + 2)
    SH1 = NCOLS0 + 2

    def din(name, shape, dt):
        return nc.dram_tensor(name, list(shape), dt, kind="ExternalInput")

    xs = din("xs", (TH0, D), bf16)
    xm = din("xm", (TH0, D), bf16)
    idx = {}
    for l, lay in enumerate(prep["layers"]):
        for h in (0, 1):
            slots = lay["geo"][h][1]
            idx[(l, h)] = din(f"idx_l{l}h{h}",
                              (slots // TILE_SLOTS, 128, TILE_SLOTS // 16), i16)
        idx[(l, "r0")] = din(f"idxr0_l{l}", (128, lay["ncols"] // 16), i16)
        idx[(l, "r1")] = din(f"idxr1_l{l}", (128, lay["ncols"] // 16), i16)
    invdeg = [din(f"invdeg_l{l}", (128, prep["layers"][l]["ncols"]), bf16)
              for l in range(2)]
    mask0 = din("mask0", (128, NCOLS0), bf16)
    maskpool = din("maskpool", (128, NCOLS1), bf16)
    wpar = {}
    for l in range(2):
        for bn in ("me", "ma"):
            wpar[f"wl_l{l}{bn}"] = din(f"wl_l{l}{bn}", (D, D), bf16)
            wpar[f"wr_l{l}{bn}"] = din(f"wr_l{l}{bn}", (D, D), bf16)
            for nm in ("bl", "bns", "bnb"):
                wpar[f"{nm}_l{l}{bn}"] = din(f"{nm}_l{l}{bn}", (D, 1), f32)
    cw1 = din("cw1", (128, 256), f32)
    cw2 = din("cw2", (128, 2, 128), f32)
    cw3 = din("cw3", (128, 2), f32)
    cs1 = din("cs1", (128, 2), f32)
    cb1 = din("cb1", (128, 2), f32)
    cs2 = din("cs2", (128, 1), f32)
    cb2 = din("cb2", (128, 1), f32)
    cb3 = din("cb3", (2, 1), f32)
    out_d = nc.dram_tensor("out", [130], f32, kind="ExternalOutput")

    with tile.TileContext(nc) as tc:
        with (
            tc.tile_pool(name="dram", bufs=1, space="DRAM") as dp,
            tc.tile_pool(name="sb", bufs=1) as sb,
            tc.tile_pool(name="ep", bufs=2) as ep,
            tc.tile_pool(name="ps", bufs=2, space="PSUM") as ps,
            tc.tile_pool(name="ps2", bufs=2, space="PSUM") as ps2,
            nc.allow_low_precision(reason="fp16 segment sums; tol 2e-2"),
        ):
            nc.gpsimd.load_library(mlp_lib)
            tab2 = dp.tile([TH1, D], bf16, addr_space="Shared")
            h1_shard = dp.tile([SH1, D], bf16)
            pool_in = dp.tile([128, 1], f32)
            pool_out = dp.tile([128, 1], f32, addr_space="Shared")

            ident = sb.tile([128, 128], bf16)
            make_identity(nc, ident[:])
            ones_col = sb.tile([128, 1], f32)
            nc.vector.memset(ones_col[:], 1.0)
            onerow = sb.tile([1, 128], f32)   # lhsT for partition-broadcast
            nc.vector.memset(onerow[:], 1.0)
            eps_t = sb.tile([1, 1], f32)
            nc.vector.memset(eps_t[:], 1e-30)
            zrow = sb.tile([1, D], bf16)
            nc.vector.memset(zrow[:], 0.0)
            nc.sync.dma_start(out=h1_shard[0:1, :], in_=zrow[:])
            nc.sync.dma_start(out=h1_shard[SH1 - 1:SH1, :], in_=zrow[:])

            for l, lay in enumerate(prep["layers"]):
                nco = lay["ncols"]
                HALF = 4 * lay["S_in"]
                tabs = (xs, xm) if l == 0 else (tab2, tab2)

                def half_ap(t, h):
                    return t[HALF * h:HALF * (h + 1), :]

                with tc.tile_pool(name=f"agg{l}", bufs=1) as aggp:
                    s_buf = aggp.tile([128, nco], bf16, name=f"s_buf{l}")
                    m_buf = aggp.tile([128, nco], bf16, name=f"m_buf{l}")
                    h_T = aggp.tile([128, nco], bf16, name=f"h_T{l}")
                    nc.vector.memset(s_buf[:], 0.0)
                    nc.vector.memset(m_buf[:], NEG if l == 0 else 0.0)

                    with tc.tile_pool(name=f"gat{l}", bufs=2) as gp, \
                         tc.tile_pool(name=f"gx{l}", bufs=1) as gx:
                        tmp_s = gx.tile([128, CHUNK], bf16, name=f"tmp_s{l}")
                        tmp_m = gx.tile([128, CHUNK], bf16, name=f"tmp_m{l}")
                        for h in (0, 1):
                            segs, slots = lay["geo"][h]
                            nt = slots // TILE_SLOTS
                            by_tile = [[] for _ in range(nt)]
                            for (k, L, take, col, cursor) in segs:
                                by_tile[cursor // TILE_SLOTS].append(
                                    (cursor % TILE_SLOTS, L, take, col))
                            for t in range(nt):
                                it = gp.tile([128, TILE_SLOTS // 16], i16)
                                nc.sync.dma_start(out=it[:], in_=idx[(l, h)][t])
                                gs = gp.tile([128, TILE_SLOTS], bf16)
                                for g0 in range(0, TILE_SLOTS, GI):
                                    nc.gpsimd.dma_gather(
                                        gs[:, g0:g0 + GI].rearrange(
                                            "p (o t) -> p o t", o=1),
                                        half_ap(tabs[0], h),
                                        it[:, g0 // 16:(g0 + GI) // 16],
                                        GI, GI, D, transpose=True)
                                if l == 0:
                                    gm = gp.tile([128, TILE_SLOTS], bf16)
                                    for g0 in range(0, TILE_SLOTS, GI):
                                        nc.gpsimd.dma_gather(
                                            gm[:, g0:g0 + GI].rearrange(
                                                "p (o t) -> p o t", o=1),
                                            half_ap(tabs[1], h),
                                            it[:, g0 // 16:(g0 + GI) // 16],
                                            GI, GI, D, transpose=True)
                                else:
                                    gm = gs
                                for (off, L, m, col) in by_tile[t]:
                                    vs = gs[:, off:off + m * L].rearrange(
                                        "p (m l) -> p m l", l=L)
                                    vm = gm[:, off:off + m * L].rearrange(
                                        "p (m l) -> p m l", l=L)
                                    if h == 0:
                                        nc.vector.tensor_reduce(
                                            s_buf[:, col:col + m], vs,
                                            mybir.AxisListType.X,
                                            mybir.AluOpType.add)
                                        nc.vector.tensor_reduce(
                                            m_buf[:, col:col + m], vm,
                                            mybir.AxisListType.X,
                                            mybir.AluOpType.max)
                                    else:
                                        nc.vector.tensor_reduce(
                                            tmp_s[:, :m], vs,
                                            mybir.AxisListType.X,
                                            mybir.AluOpType.add)
                                        nc.vector.tensor_tensor(
                                            out=s_buf[:, col:col + m],
                                            in0=s_buf[:, col:col + m],
                                            in1=tmp_s[:, :m],
                                            op=mybir.AluOpType.add)
                                        nc.vector.tensor_reduce(
                                            tmp_m[:, :m], vm,
                                            mybir.AxisListType.X,
                                            mybir.AluOpType.max)
                                        nc.vector.tensor_tensor(
                                            out=m_buf[:, col:col + m],
                                            in0=m_buf[:, col:col + m],
                                            in1=tmp_m[:, :m],
                                            op=mybir.AluOpType.max)
                        # lin_r input (two half-gathers, summed in place)
                        ita = gx.tile([128, nco // 16], i16, name=f"ita{l}")
                        itb = gx.tile([128, nco // 16], i16, name=f"itb{l}")
                        nc.sync.dma_start(out=ita[:], in_=idx[(l, "r0")][:, :])
                        nc.sync.dma_start(out=itb[:], in_=idx[(l, "r1")][:, :])
                        hTb = gx.tile([128, nco], bf16, name=f"hTb{l}")
                        for g0 in range(0, nco, GI):
                            gi = min(GI, nco - g0)
                            nc.gpsimd.dma_gather(
                                h_T[:, g0:g0 + gi].rearrange(
                                    "p (o t) -> p o t", o=1),
                                half_ap(tabs[0], 0),
                                ita[:, g0 // 16:(g0 + gi) // 16],
                                gi, gi, D, transpose=True)
                            nc.gpsimd.dma_gather(
                                hTb[:, g0:g0 + gi].rearrange(
                                    "p (o t) -> p o t", o=1),
                                half_ap(tabs[0], 1),
                                itb[:, g0 // 16:(g0 + gi) // 16],
                                gi, gi, D, transpose=True)
                        nc.vector.tensor_tensor(out=h_T[:], in0=h_T[:],
                                                in1=hTb[:],
                                                op=mybir.AluOpType.add)
                        if STAGE == 1:
                            _dump_exit(nc, gx, out_d, s_buf[:, 0:1])
                            return

                    with tc.tile_pool(name=f"epi{l}", bufs=1) as epi:
                        aux = epi.tile([128, nco], bf16, name=f"aux{l}")
                        nc.sync.dma_start(out=aux[:], in_=invdeg[l][:, :])
                        nc.vector.tensor_tensor(out=s_buf[:], in0=s_buf[:],
                                                in1=aux[:],
                                                op=mybir.AluOpType.mult)
                        if l == 0:
                            nc.sync.dma_start(out=aux[:], in_=mask0[:, :])
                            nc.vector.tensor_tensor(out=m_buf[:], in0=m_buf[:],
                                                    in1=aux[:],
                                                    op=mybir.AluOpType.mult)
                        # s_buf now holds the mean aggregate
                        hb0 = epi.tile([128, nco], bf16, name=f"hb0{l}")
                        hb1 = epi.tile([128, nco], bf16, name=f"hb1{l}")
                        t_f = epi.tile([128, nco], f32, name=f"t_f{l}")
                        n2 = epi.tile([1, nco], f32, name=f"n2{l}")   # also reused for sqrt/recip in-place
                        nt512 = -(-nco // 512)
                        for bn, agg, hb in (("me", s_buf, hb0),
                                            ("ma", m_buf, hb1)):
                            wl = sb.tile([128, 128], bf16, name=f"wl{l}{bn}")
                            wr = sb.tile([128, 128], bf16, name=f"wr{l}{bn}")
                            nc.sync.dma_start(out=wl[:],
                                              in_=wpar[f"wl_l{l}{bn}"][:, :])
                            nc.sync.dma_start(out=wr[:],
                                              in_=wpar[f"wr_l{l}{bn}"][:, :])
                            blv = sb.tile([128, 1], f32, name=f"bl{l}{bn}")
                            bns = sb.tile([128, 1], f32, name=f"bns{l}{bn}")
                            bnb = sb.tile([128, 1], f32, name=f"bnb{l}{bn}")
                            nc.sync.dma_start(out=blv[:],
                                              in_=wpar[f"bl_l{l}{bn}"][:, :])
                            nc.sync.dma_start(out=bns[:],
                                              in_=wpar[f"bns_l{l}{bn}"][:, :])
                            nc.sync.dma_start(out=bnb[:],
                                              in_=wpar[f"bnb_l{l}{bn}"][:, :])
                            for tt in range(nt512):
                                c0 = tt * 512
                                c1 = min(c0 + 512, nco)
                                w = c1 - c0
                                pt = ps.tile([128, 512], f32)
                                nc.tensor.matmul(pt[:, :w], lhsT=wl[:],
                                                 rhs=agg[:, c0:c1],
                                                 start=True, stop=False)
                                nc.tensor.matmul(pt[:, :w], lhsT=wr[:],
                                                 rhs=h_T[:, c0:c1],
                                                 start=False, stop=True)
                                nc.vector.tensor_scalar(
                                    out=t_f[:, c0:c1], in0=pt[:, :w],
                                    scalar1=blv[:], scalar2=None,
                                    op0=mybir.AluOpType.add)
                                sq = ep.tile([128, 512], f32)
                                nc.vector.tensor_tensor(
                                    out=sq[:, :w], in0=t_f[:, c0:c1],
                                    in1=t_f[:, c0:c1],
                                    op=mybir.AluOpType.mult)
                                p2 = ps2.tile([1, 512], f32)
                                nc.tensor.matmul(p2[:, :w], lhsT=ones_col[:],
                                                 rhs=sq[:, :w],
                                                 start=True, stop=True)
                                nc.vector.tensor_copy(out=n2[:, c0:c1],
                                                      in_=p2[:, :w])
                            nc.scalar.activation(
                                n2[:], n2[:],
                                mybir.ActivationFunctionType.Sqrt,
                                bias=eps_t[:], scale=1.0)
                            rinv = n2
                            nc.vector.reciprocal(rinv[:], rinv[:])
                            for tt in range(nt512):
                                c0 = tt * 512
                                c1 = min(c0 + 512, nco)
                                w = c1 - c0
                                rep = ps.tile([128, 512], f32)
                                nc.tensor.matmul(rep[:, :w], lhsT=onerow[:],
                                                 rhs=rinv[:, c0:c1],
                                                 start=True, stop=True)
                                z = ep.tile([128, 512], f32)
                                nc.vector.tensor_tensor(
                                    out=z[:, :w], in0=t_f[:, c0:c1],
                                    in1=rep[:, :w], op=mybir.AluOpType.mult)
                                nc.scalar.activation(
                                    hb[:, c0:c1], z[:, :w],
                                    mybir.ActivationFunctionType.Relu,
                                    bias=bnb[:], scale=bns[:])
                        h_new = hb0
                        nc.vector.tensor_tensor(out=h_new[:], in0=hb0[:],
                                                in1=hb1[:],
                                                op=mybir.AluOpType.add)
                        if STAGE == 2 and l == 0:
                            _dump_exit(nc, epi, out_d, h_new[:, 0:1])
                            return
                        if STAGE == 4 and l == 1:
                            _dump_exit(nc, epi, out_d, h_new[:, 0:1])
                            return

                        if l == 0:
                            hnm = epi.tile([128, nco // 128, 128], bf16,
                                           name="hnm")
                            for b in range(nco // 128):
                                pt = ps.tile([128, 128], bf16)
                                nc.tensor.transpose(
                                    out=pt[:],
                                    in_=h_new[:, b * 128:(b + 1) * 128],
                                    identity=ident[:])
                                nc.vector.tensor_copy(out=hnm[:, b, :],
                                                      in_=pt[:])
                            for b in range(nco // 128):
                                nc.sync.dma_start(
                                    out=h1_shard[1 + b * 128:
                                                 1 + (b + 1) * 128, :],
                                    in_=hnm[:, b, :])
                            nc.gpsimd.collective_compute(
                                "AllGather", mybir.AluOpType.bypass,
                                replica_groups=[list(range(NC))],
                                ins=[h1_shard[:, :].opt()],
                                outs=[tab2[:, :].opt()],
                            )
                            if STAGE == 3:
                                tg = epi.tile([128, 1], bf16, name="dbgtg")
                                nc.sync.dma_start(
                                    out=tg[:],
                                    in_=tab2[1:129, 0:1])
                                _dump_exit(nc, epi, out_d, tg[:])
                                return
                        else:
                            nc.sync.dma_start(out=aux[:], in_=maskpool[:, :])
                            nc.vector.tensor_tensor(out=h_new[:],
                                                    in0=h_new[:], in1=aux[:],
                                                    op=mybir.AluOpType.mult)
                            pool_sb = epi.tile([128, 1], f32, name="pool_sb")
                            nc.vector.tensor_reduce(pool_sb[:], h_new[:],
                                                    mybir.AxisListType.X,
                                                    mybir.AluOpType.add)
                            nc.sync.dma_start(out=pool_in[:, :],
                                              in_=pool_sb[:])
                            nc.gpsimd.collective_compute(
                                "AllReduce", mybir.AluOpType.add,
                                replica_groups=[list(range(NC))],
                                ins=[pool_in[:, :].opt()],
                                outs=[pool_out[:, :].opt()],
                            )
                            emb = epi.tile([128, 1], f32, name="emb")
                            nc.sync.dma_start(out=emb[:], in_=pool_out[:, :])
                            embm = epi.tile([128, 1], f32, name="embm")
                            nc.vector.tensor_scalar(
                                out=embm[:], in0=emb[:], scalar1=1.0 / N,
                                scalar2=None, op0=mybir.AluOpType.mult)
                            nc.sync.dma_start(
                                out=out_d[2:130].rearrange("(x o) -> x o",
                                                           o=1),
                                in_=embm[:])
                            _classifier(nc, sb, epi, ps2, embm, cw1, cw2,
                                        cw3, cs1, cb1, cs2, cb2, cb3, out_d)


def _classifier(nc, sb, wp, ps2, z0, cw1, cw2, cw3, cs1, cb1, cs2, cb2,
                cb3, out_d):
    f32 = mybir.dt.float32
    w1 = sb.tile([128, 256], f32)
    w2 = sb.tile([128, 2, 128], f32)
    w3 = sb.tile([128, 2], f32)
    nc.sync.dma_start(out=w1[:], in_=cw1[:, :])
    nc.sync.dma_start(out=w2[:], in_=cw2[:, :, :])
    nc.sync.dma_start(out=w3[:], in_=cw3[:, :])
    s1 = sb.tile([128, 2], f32)
    b1 = sb.tile([128, 2], f32)
    s2 = sb.tile([128, 1], f32)
    b2 = sb.tile([128, 1], f32)
    b3 = sb.tile([2, 1], f32)
    nc.sync.dma_start(out=s1[:], in_=cs1[:, :])
    nc.sync.dma_start(out=b1[:], in_=cb1[:, :])
    nc.sync.dma_start(out=s2[:], in_=cs2[:, :])
    nc.sync.dma_start(out=b2[:], in_=cb2[:, :])
    nc.sync.dma_start(out=b3[:], in_=cb3[:, :])
    z1 = wp.tile([128, 2], f32, name="z1")
    for hh in (0, 1):
        ptc = ps2.tile([128, 1], f32, name="csps")
        nc.tensor.matmul(ptc[:], lhsT=w1[:, hh * 128:(hh + 1) * 128],
                         rhs=z0[:], start=True, stop=True)
        nc.scalar.activation(z1[:, hh:hh + 1], ptc[:],
                             mybir.ActivationFunctionType.Relu,
                             bias=b1[:, hh:hh + 1], scale=s1[:, hh:hh + 1])
    pt2 = ps2.tile([128, 1], f32, name="csps")
    nc.tensor.matmul(pt2[:], lhsT=w2[:, 0, :], rhs=z1[:, 0:1],
                     start=True, stop=False)
    nc.tensor.matmul(pt2[:], lhsT=w2[:, 1, :], rhs=z1[:, 1:2],
                     start=False, stop=True)
    z2 = wp.tile([128, 1], f32, name="z2")
    nc.scalar.activation(z2[:], pt2[:], mybir.ActivationFunctionType.Relu,
                         bias=b2[:], scale=s2[:])
    pt3 = ps2.tile([2, 1], f32, name="csps")
    nc.tensor.matmul(pt3[:], lhsT=w3[:], rhs=z2[:], start=True, stop=True)
    lg = wp.tile([2, 1], f32, name="lg")
    nc.vector.tensor_tensor(out=lg[:], in0=pt3[:], in1=b3[:],
                            op=mybir.AluOpType.add)
    nc.sync.dma_start(out=out_d[0:2].rearrange("(x o) -> x o", o=1),
                      in_=lg[:])


# ================================================================== entry

def kernel(x, edge_index, params):
    x = np.asarray(x, np.float32)
    prep, per_core = host_prep(x, edge_index, params)
    nc = bacc.Bacc(get_trn_type() or "TRN2", target_bir_lowering=False,
                   debug=False, num_devices=NC)
    build(nc, prep)
    nc.compile()
    import os
    trace = bool(int(os.environ.get("BASS_KERNEL_TRACE", "0")))
    res = run_bass_kernel_spmd(nc, per_core, core_ids=list(range(NC)),
                               trace=trace)
    if trace:
        print(f"HW exec time: {res.exec_time_ns} ns")
        print(f"profile: {res.profile_json}")
    out = np.asarray(res.results[0]["out"], np.float32)
    return out[:2].reshape(1, 2).copy(), out[2:130].reshape(1, 128).copy()
